# revision 7
# baseline (speedup 1.0000x reference)
"""Multi-head attention (B=4, S=2048, D=768, H=12) on 8 TRN2 NeuronCores.

Sharding: 48 (batch, head) units -> core c handles batch c//2, heads
6*(c%2) .. 6*(c%2)+5 (tensor-parallel over heads). Each core computes a
partial output projection; the host sums the two partials per batch and
adds the bias.

Projections and the output projection run in float32r (fp32 storage, PE
rounds to ~12 mantissa bits, full speed at N>=256); the attention
matmuls (logits, attn@V) run in bf16, which streams 29% faster through
the PE. Inputs are pre-rounded and pre-transposed on the host so the
device never transposes.
"""

import numpy as np

import concourse.bacc as bacc
import concourse.mybir as mybir
from concourse import tile
from concourse.bass_utils import run_bass_kernel_spmd

B, S, D, H = 4, 2048, 768, 12
DEPTH = D // H  # 64
HPC = H // 2  # heads per core: 6
HD = HPC * DEPTH  # per-core projected dim: 384
EC = D // 128  # e chunks: 6
MT = HD // 128  # d tiles: 3
ST = S // 128  # s tiles: 16
SH = 2  # s halves for projection rhs streaming
QH = 2  # q halves in attention
QHS = S // QH  # 1024

f32 = mybir.dt.float32
f32r = mybir.dt.float32r
bf16 = mybir.dt.bfloat16
AF = mybir.ActivationFunctionType

_CACHE = {}


def _build():
    if "nc" in _CACHE:
        return _CACHE["nc"]
    nc = bacc.Bacc("TRN2", target_bir_lowering=False, debug=False, num_devices=8)
    qt = nc.dram_tensor("qt", [D, S], f32r, kind="ExternalInput").ap()
    kt = nc.dram_tensor("kt", [D, S], f32r, kind="ExternalInput").ap()
    vt = nc.dram_tensor("vt", [D, S], f32r, kind="ExternalInput").ap()
    wqt = nc.dram_tensor("wqt", [D, HD], f32r, kind="ExternalInput").ap()
    wkt = nc.dram_tensor("wkt", [D, HD], f32r, kind="ExternalInput").ap()
    wvt = nc.dram_tensor("wvt", [D, HD], f32r, kind="ExternalInput").ap()
    wot = nc.dram_tensor("wot", [HD, D], f32r, kind="ExternalInput").ap()
    y = nc.dram_tensor("y", [S, D], f32, kind="ExternalOutput").ap()

    with tile.TileContext(nc) as tc:
        with (
            tc.tile_pool(name="wp", bufs=3) as wp,
            tc.tile_pool(name="wop", bufs=1) as wop,
            tc.tile_pool(name="xp", bufs=8) as xp,
            tc.tile_pool(name="qk", bufs=2 * MT) as qkp,
            tc.tile_pool(name="vg", bufs=ST) as vgp,
            tc.tile_pool(name="ot", bufs=MT) as otp,
            tc.tile_pool(name="ep", bufs=4) as epp,
            tc.tile_pool(name="sm", bufs=2) as smp,
            tc.tile_pool(name="yp", bufs=2) as ypp,
        ):
            # ---- persistent SBUF tensors ----
            qht = [qkp.tile([128, S], f32r, tag="qk", name=f"qht{i}") for i in range(MT)]
            kht = [qkp.tile([128, S], f32r, tag="qk", name=f"kht{i}") for i in range(MT)]
            vaug = [vgp.tile([128, HPC, DEPTH + 1], bf16, tag="vg", name=f"vaug{i}") for i in range(ST)]
            outt = [otp.tile([128, S], f32r, tag="ot", name=f"outt{i}") for i in range(MT)]

            wot_sb = wop.tile([128, MT, D], f32r, tag="wot")
            nc.sync.dma_start(
                out=wot_sb[:], in_=wot.rearrange("(m p) o -> p m o", p=128)
            )

            def load_w(wdram, nm):
                w_sb = wp.tile([128, EC, HD], f32r, tag="w", name=f"w_{nm}")
                for ci in range(EC):
                    nc.sync.dma_start(
                        out=w_sb[:, ci, :],
                        in_=wdram[ci * 128 : (ci + 1) * 128, :],
                    )
                return w_sb

            def load_x(xdram, sh, nm):
                xc = [
                    xp.tile([128, S // SH], f32r, tag="x", name=f"x{nm}{sh}_{i}")
                    for i in range(EC)
                ]
                for ci in range(EC):
                    nc.sync.dma_start(
                        out=xc[ci][:],
                        in_=xdram[
                            ci * 128 : (ci + 1) * 128,
                            sh * (S // SH) : (sh + 1) * (S // SH),
                        ],
                    )
                return xc

            with (
                tc.tile_pool(name="plog", bufs=2, space="PSUM") as plog,
                tc.tile_pool(name="pacc", bufs=2, space="PSUM") as pacc,
            ):
                # ---- phase emitters ----
                def proj_qk(name, xdram, w_sb, dst, m):
                    # one d-tile (m) of a Q/K projection; reloads x chunks
                    for sh in range(SH):
                        xc = load_x(xdram, sh, f"{name}{m}")
                        pt = plog.tile(
                            [128, S // SH], f32, tag="plog", name=f"p{name}{sh}_{m}"
                        )
                        for ci in range(EC):
                            for n in range(S // SH // 512):
                                nc.tensor.matmul(
                                    pt[:, n * 512 : (n + 1) * 512],
                                    w_sb[:, ci, m * 128 : (m + 1) * 128],
                                    xc[ci][:, n * 512 : (n + 1) * 512],
                                    start=(ci == 0),
                                    stop=(ci == EC - 1),
                                )
                        with nc.allow_low_precision(reason="f32r round"):
                            nc.vector.tensor_copy(
                                dst[m][:, sh * (S // SH) : (sh + 1) * (S // SH)],
                                pt[:],
                            )

                def proj_v():
                    wv_sb = load_w(wvt, "v")
                    for sh in range(SH):
                        xc = load_x(vt, sh, "v")
                        for st in range(ST // SH):
                            s = sh * (ST // SH) + st
                            pv = pacc.tile([128, HD], f32, tag="pacc", name=f"pv{s}")
                            for ci in range(EC):
                                nc.tensor.matmul(
                                    pv[:],
                                    xc[ci][:, st * 128 : (st + 1) * 128],
                                    wv_sb[:, ci, :],
                                    start=(ci == 0),
                                    stop=(ci == EC - 1),
                                )
                            with nc.allow_low_precision(reason="bf16 attention"):
                                nc.vector.tensor_copy(
                                    vaug[s][:, :, 0:DEPTH],
                                    pv[:].rearrange("p (h d) -> p h d", d=DEPTH),
                                )
                            nc.vector.memset(vaug[s][:, :, DEPTH : DEPTH + 1], 1.0)

                def attn(h, qh):
                    m = h // 2
                    base = (h % 2) * 64
                    q0 = qh * QHS
                    acc = pacc.tile(
                        [DEPTH + 1, QHS], f32, tag="pacc", name=f"acc{h}_{qh}"
                    )
                    for kt_i in range(ST):
                        lp = plog.tile(
                            [128, QHS], f32, tag="plog", name=f"lp{h}_{qh}_{kt_i}"
                        )
                        for n in range(QHS // 512):
                            nc.tensor.matmul(
                                lp[:, n * 512 : (n + 1) * 512],
                                kht[m][
                                    base : base + 64,
                                    kt_i * 128 : (kt_i + 1) * 128,
                                ],
                                qht[m][
                                    base : base + 64,
                                    q0 + n * 512 : q0 + (n + 1) * 512,
                                ],
                                start=True,
                                stop=True,
                            )
                        et = epp.tile(
                            [128, QHS], bf16, tag="ep", name=f"et{h}_{qh}_{kt_i}"
                        )
                        with nc.allow_low_precision(reason="bf16 attention"):
                            nc.scalar.activation(
                                et[:], lp[:], AF.Exp, scale=1.0 / np.sqrt(DEPTH)
                            )
                        for n in range(QHS // 512):
                            nc.tensor.matmul(
                                acc[:, n * 512 : (n + 1) * 512],
                                vaug[kt_i][:, h, :],
                                et[:, n * 512 : (n + 1) * 512],
                                start=(kt_i == 0),
                                stop=(kt_i == ST - 1),
                            )
                    r = smp.tile([1, QHS], f32r, tag="r", name=f"r{h}_{qh}")
                    with nc.allow_low_precision(reason="f32r round"):
                        nc.vector.reciprocal(r[:], acc[DEPTH : DEPTH + 1, :])
                    rb = smp.tile([64, QHS], f32r, tag="rb", name=f"rb{h}_{qh}")
                    nc.gpsimd.partition_broadcast(rb[:], r[:])
                    with nc.allow_low_precision(reason="f32r round"):
                        nc.vector.tensor_mul(
                            outt[m][base : base + 64, q0 : q0 + QHS],
                            acc[0:DEPTH, :],
                            rb[:].bitcast(f32),
                        )

                def outproj(s):
                    py = pacc.tile([128, D], f32, tag="pacc", name=f"py{s}")
                    for m in range(MT):
                        for n0, n1 in ((0, 512), (512, 768)):
                            nc.tensor.matmul(
                                py[:, n0:n1],
                                outt[m][:, s * 128 : (s + 1) * 128],
                                wot_sb[:, m, n0:n1],
                                start=(m == 0),
                                stop=(m == MT - 1),
                            )
                    ty = ypp.tile([128, D], f32, tag="y", name=f"ty{s}")
                    nc.vector.tensor_copy(ty[:], py[:])
                    nc.sync.dma_start(out=y[s * 128 : (s + 1) * 128, :], in_=ty[:])

                # ---- interleaved emission ----
                wq_sb = load_w(wqt, "q")
                wk_sb = load_w(wkt, "k")
                proj_qk("q", qt, wq_sb, qht, 0)
                proj_qk("k", kt, wk_sb, kht, 0)
                proj_v()
                attn(0, 0)
                attn(1, 0)
                proj_qk("q", qt, wq_sb, qht, 1)
                proj_qk("k", kt, wk_sb, kht, 1)
                attn(2, 0)
                attn(3, 0)
                proj_qk("q", qt, wq_sb, qht, 2)
                proj_qk("k", kt, wk_sb, kht, 2)
                attn(4, 0)
                attn(5, 0)
                for s in range(ST // 2):
                    outproj(s)
                for h in range(HPC):
                    attn(h, 1)
                for s in range(ST // 2, ST):
                    outproj(s)

    nc.compile()
    _CACHE["nc"] = nc
    return nc


def _round_f32r(x: np.ndarray) -> np.ndarray:
    b = np.ascontiguousarray(x, dtype=np.float32).view(np.uint32).astype(np.int64)
    r = (b + 0x800 + ((b >> 12) & 1)) & ~0xFFF
    return r.astype(np.uint32).view(np.float32)


def make_in_maps(v, k, q, wq, wk, wv, wo):
    in_maps = []
    for c in range(8):
        b = c // 2
        hs = (c % 2) * HD
        in_maps.append(
            {
                "qt": _round_f32r(q[b].T),
                "kt": _round_f32r(k[b].T),
                "vt": _round_f32r(v[b].T),
                "wqt": _round_f32r(wq[hs : hs + HD, :].T),
                "wkt": _round_f32r(wk[hs : hs + HD, :].T),
                "wvt": _round_f32r(wv[hs : hs + HD, :].T),
                "wot": _round_f32r(wo[:, hs : hs + HD].T),
            }
        )
    return in_maps


def assemble(results, bo):
    y = np.empty((B, S, D), dtype=np.float32)
    for b in range(B):
        y[b] = results[2 * b]["y"] + results[2 * b + 1]["y"] + bo[None, :]
    return y


def kernel(v, k, q, wq, wk, wv, wo, bo):
    nc = _build()
    in_maps = make_in_maps(v, k, q, wq, wk, wv, wo)
    res = run_bass_kernel_spmd(nc, in_maps, list(range(8)))
    return assemble(res.results, np.asarray(bo, dtype=np.float32))


# revision 8
# speedup vs baseline: 1.0377x; 1.0377x over previous
"""Multi-head attention (B=4, S=2048, D=768, H=12) on 8 TRN2 NeuronCores.

Sharding: 48 (batch, head) units -> core c handles batch c//2, heads
6*(c%2) .. 6*(c%2)+5 (tensor-parallel over heads). Each core computes a
partial output projection; the host sums the two partials per batch and
adds the bias.

Q/K/V projections run in float32r (fp32 storage, PE rounds to ~12
mantissa bits, full speed at N>=256). Attention (logits, exp, attn@V)
and the output projection run in fp16 (10 mantissa bits - 8x more
precise than bf16 at the same PE speed, half the SBUF traffic of f32r).
Inputs are pre-rounded and pre-transposed on the host so the device
never transposes. End-to-end rel err vs fp64 is ~5e-4.
"""

import numpy as np

import concourse.bacc as bacc
import concourse.mybir as mybir
from concourse import tile
from concourse.bass_utils import run_bass_kernel_spmd

B, S, D, H = 4, 2048, 768, 12
DEPTH = D // H  # 64
HPC = H // 2  # heads per core: 6
HD = HPC * DEPTH  # per-core projected dim: 384
EC = D // 128  # e chunks: 6
MT = HD // 128  # d tiles: 3
ST = S // 128  # s tiles: 16
SH = 2  # s halves for projection rhs streaming
QH = 2  # q halves in attention
QHS = S // QH  # 1024

f32 = mybir.dt.float32
f32r = mybir.dt.float32r
fp16 = mybir.dt.float16
AF = mybir.ActivationFunctionType

_CACHE = {}


def _build():
    if "nc" in _CACHE:
        return _CACHE["nc"]
    nc = bacc.Bacc("TRN2", target_bir_lowering=False, debug=False, num_devices=8)
    qt = nc.dram_tensor("qt", [D, S], f32r, kind="ExternalInput").ap()
    kt = nc.dram_tensor("kt", [D, S], f32r, kind="ExternalInput").ap()
    vt = nc.dram_tensor("vt", [D, S], f32r, kind="ExternalInput").ap()
    wqt = nc.dram_tensor("wqt", [D, HD], f32r, kind="ExternalInput").ap()
    wkt = nc.dram_tensor("wkt", [D, HD], f32r, kind="ExternalInput").ap()
    wvt = nc.dram_tensor("wvt", [D, HD], f32r, kind="ExternalInput").ap()
    wot = nc.dram_tensor("wot", [HD, D], fp16, kind="ExternalInput").ap()
    y = nc.dram_tensor("y", [S, D], f32, kind="ExternalOutput").ap()

    with tile.TileContext(nc) as tc:
        with (
            tc.tile_pool(name="wp", bufs=3) as wp,
            tc.tile_pool(name="wop", bufs=1) as wop,
            tc.tile_pool(name="xp", bufs=8) as xp,
            tc.tile_pool(name="qk", bufs=2 * MT) as qkp,
            tc.tile_pool(name="vg", bufs=ST) as vgp,
            tc.tile_pool(name="ot", bufs=MT) as otp,
            tc.tile_pool(name="ep", bufs=8) as epp,
            tc.tile_pool(name="sm", bufs=2) as smp,
            tc.tile_pool(name="yp", bufs=2) as ypp,
        ):
            # ---- persistent SBUF tensors ----
            qht = [qkp.tile([128, S], fp16, tag="qk", name=f"qht{i}") for i in range(MT)]
            kht = [qkp.tile([128, S], fp16, tag="qk", name=f"kht{i}") for i in range(MT)]
            vaug = [vgp.tile([128, HPC, DEPTH + 1], fp16, tag="vg", name=f"vaug{i}") for i in range(ST)]
            outt = [otp.tile([128, S], fp16, tag="ot", name=f"outt{i}") for i in range(MT)]

            wot_sb = wop.tile([128, MT, D], fp16, tag="wot")
            nc.sync.dma_start(
                out=wot_sb[:], in_=wot.rearrange("(m p) o -> p m o", p=128)
            )

            def load_w(wdram, nm):
                w_sb = wp.tile([128, EC, HD], f32r, tag="w", name=f"w_{nm}")
                for ci in range(EC):
                    nc.sync.dma_start(
                        out=w_sb[:, ci, :],
                        in_=wdram[ci * 128 : (ci + 1) * 128, :],
                    )
                return w_sb

            def load_x(xdram, sh, nm):
                xc = [
                    xp.tile([128, S // SH], f32r, tag="x", name=f"x{nm}{sh}_{i}")
                    for i in range(EC)
                ]
                for ci in range(EC):
                    nc.sync.dma_start(
                        out=xc[ci][:],
                        in_=xdram[
                            ci * 128 : (ci + 1) * 128,
                            sh * (S // SH) : (sh + 1) * (S // SH),
                        ],
                    )
                return xc

            with (
                tc.tile_pool(name="plog", bufs=2, space="PSUM") as plog,
                tc.tile_pool(name="pacc", bufs=2, space="PSUM") as pacc,
            ):

                def proj_qk(name, xdram, w_sb, dst, m):
                    # one d-tile (m) of a Q/K projection (reloads x halves)
                    for sh in range(SH):
                        xc = load_x(xdram, sh, f"{name}{m}")
                        pt = plog.tile(
                            [128, S // SH], f32, tag="plog", name=f"p{name}{sh}_{m}"
                        )
                        for ci in range(EC):
                            for n in range(S // SH // 512):
                                nc.tensor.matmul(
                                    pt[:, n * 512 : (n + 1) * 512],
                                    w_sb[:, ci, m * 128 : (m + 1) * 128],
                                    xc[ci][:, n * 512 : (n + 1) * 512],
                                    start=(ci == 0),
                                    stop=(ci == EC - 1),
                                )
                        with nc.allow_low_precision(reason="fp16 attention"):
                            nc.vector.tensor_copy(
                                dst[m][:, sh * (S // SH) : (sh + 1) * (S // SH)],
                                pt[:],
                            )

                def proj_v():
                    wv_sb = load_w(wvt, "v")
                    for sh in range(SH):
                        xc = load_x(vt, sh, "v")
                        for st in range(ST // SH):
                            s = sh * (ST // SH) + st
                            pv = pacc.tile([128, HD], f32, tag="pacc", name=f"pv{s}")
                            for ci in range(EC):
                                nc.tensor.matmul(
                                    pv[:],
                                    xc[ci][:, st * 128 : (st + 1) * 128],
                                    wv_sb[:, ci, :],
                                    start=(ci == 0),
                                    stop=(ci == EC - 1),
                                )
                            with nc.allow_low_precision(reason="fp16 attention"):
                                nc.vector.tensor_copy(
                                    vaug[s][:, :, 0:DEPTH],
                                    pv[:].rearrange("p (h d) -> p h d", d=DEPTH),
                                )
                            nc.vector.memset(vaug[s][:, :, DEPTH : DEPTH + 1], 1.0)

                def attn(h, qh):
                    m = h // 2
                    base = (h % 2) * 64
                    q0 = qh * QHS
                    acc = pacc.tile(
                        [DEPTH + 1, QHS], f32, tag="pacc", name=f"acc{h}_{qh}"
                    )
                    for kt_i in range(ST):
                        lp = plog.tile(
                            [128, QHS], f32, tag="plog", name=f"lp{h}_{qh}_{kt_i}"
                        )
                        for n in range(QHS // 512):
                            nc.tensor.matmul(
                                lp[:, n * 512 : (n + 1) * 512],
                                kht[m][
                                    base : base + 64,
                                    kt_i * 128 : (kt_i + 1) * 128,
                                ],
                                qht[m][
                                    base : base + 64,
                                    q0 + n * 512 : q0 + (n + 1) * 512,
                                ],
                                start=True,
                                stop=True,
                            )
                        et = epp.tile(
                            [128, QHS], fp16, tag="ep", name=f"et{h}_{qh}_{kt_i}"
                        )
                        with nc.allow_low_precision(reason="fp16 attention"):
                            nc.scalar.activation(
                                et[:], lp[:], AF.Exp, scale=1.0 / np.sqrt(DEPTH)
                            )
                        for n in range(QHS // 512):
                            nc.tensor.matmul(
                                acc[:, n * 512 : (n + 1) * 512],
                                vaug[kt_i][:, h, :],
                                et[:, n * 512 : (n + 1) * 512],
                                start=(kt_i == 0),
                                stop=(kt_i == ST - 1),
                            )
                    r = smp.tile([1, QHS], f32, tag="r", name=f"r{h}_{qh}")
                    nc.vector.reciprocal(r[:], acc[DEPTH : DEPTH + 1, :])
                    rb = smp.tile([64, QHS], f32, tag="rb", name=f"rb{h}_{qh}")
                    nc.gpsimd.partition_broadcast(rb[:], r[:])
                    with nc.allow_low_precision(reason="fp16 attention"):
                        nc.vector.tensor_mul(
                            outt[m][base : base + 64, q0 : q0 + QHS],
                            acc[0:DEPTH, :],
                            rb[:],
                        )

                def outproj(s):
                    py = pacc.tile([128, D], f32, tag="pacc", name=f"py{s}")
                    for m in range(MT):
                        for n0, n1 in ((0, 512), (512, 768)):
                            nc.tensor.matmul(
                                py[:, n0:n1],
                                outt[m][:, s * 128 : (s + 1) * 128],
                                wot_sb[:, m, n0:n1],
                                start=(m == 0),
                                stop=(m == MT - 1),
                            )
                    ty = ypp.tile([128, D], f32, tag="y", name=f"ty{s}")
                    nc.vector.tensor_copy(ty[:], py[:])
                    nc.sync.dma_start(out=y[s * 128 : (s + 1) * 128, :], in_=ty[:])

                # ---- interleaved emission: get ACT (exp) busy ASAP ----
                wq_sb = load_w(wqt, "q")
                wk_sb = load_w(wkt, "k")
                proj_qk("q", qt, wq_sb, qht, 0)
                proj_qk("k", kt, wk_sb, kht, 0)
                proj_v()
                attn(0, 0)
                attn(0, 1)
                attn(1, 0)
                attn(1, 1)
                proj_qk("q", qt, wq_sb, qht, 1)
                proj_qk("k", kt, wk_sb, kht, 1)
                attn(2, 0)
                attn(2, 1)
                attn(3, 0)
                attn(3, 1)
                proj_qk("q", qt, wq_sb, qht, 2)
                proj_qk("k", kt, wk_sb, kht, 2)
                attn(4, 0)
                attn(4, 1)
                attn(5, 0)
                attn(5, 1)
                for s in range(ST):
                    outproj(s)

    nc.compile()
    _CACHE["nc"] = nc
    return nc


def _round_f32r(x: np.ndarray) -> np.ndarray:
    b = np.ascontiguousarray(x, dtype=np.float32).view(np.uint32).astype(np.int64)
    r = (b + 0x800 + ((b >> 12) & 1)) & ~0xFFF
    return r.astype(np.uint32).view(np.float32)


def make_in_maps(v, k, q, wq, wk, wv, wo):
    in_maps = []
    for c in range(8):
        b = c // 2
        hs = (c % 2) * HD
        in_maps.append(
            {
                "qt": _round_f32r(q[b].T),
                "kt": _round_f32r(k[b].T),
                "vt": _round_f32r(v[b].T),
                "wqt": _round_f32r(wq[hs : hs + HD, :].T),
                "wkt": _round_f32r(wk[hs : hs + HD, :].T),
                "wvt": _round_f32r(wv[hs : hs + HD, :].T),
                "wot": np.ascontiguousarray(
                    wo[:, hs : hs + HD].T, dtype=np.float32
                ).astype(np.float16),
            }
        )
    return in_maps


def assemble(results, bo):
    y = np.empty((B, S, D), dtype=np.float32)
    for b in range(B):
        y[b] = results[2 * b]["y"] + results[2 * b + 1]["y"] + bo[None, :]
    return y


def kernel(v, k, q, wq, wk, wv, wo, bo):
    nc = _build()
    in_maps = make_in_maps(v, k, q, wq, wk, wv, wo)
    res = run_bass_kernel_spmd(nc, in_maps, list(range(8)))
    return assemble(res.results, np.asarray(bo, dtype=np.float32))


# revision 9
# speedup vs baseline: 1.3253x; 1.2772x over previous
"""Multi-head attention (B=4, S=2048, D=768, H=12) on 8 TRN2 NeuronCores.

Sharding: 48 (batch, head) units -> core c handles batch c//2, heads
6*(c%2) .. 6*(c%2)+5 (tensor-parallel over heads). Each core computes a
partial output projection; the host sums the two partials per batch and
adds the bias.

Q/K/V projections run in float32r (fp32 storage, PE rounds to ~12
mantissa bits, full speed at N>=256). Attention (logits, exp, attn@V)
and the output projection run in fp16 (10 mantissa bits - 8x more
precise than bf16 at the same PE speed, half the SBUF traffic of f32r).
Inputs are pre-rounded and pre-transposed on the host so the device
never transposes. End-to-end rel err vs fp64 is ~5e-4.
"""

import numpy as np

import concourse.bacc as bacc
import concourse.mybir as mybir
from concourse import tile
from concourse.bass_utils import run_bass_kernel_spmd

B, S, D, H = 4, 2048, 768, 12
DEPTH = D // H  # 64
HPC = H // 2  # heads per core: 6
HD = HPC * DEPTH  # per-core projected dim: 384
EC = D // 128  # e chunks: 6
MT = HD // 128  # d tiles: 3
ST = S // 128  # s tiles: 16
SH = 2  # s halves for projection rhs streaming
QH = 2  # q halves in attention
QHS = S // QH  # 1024

f32 = mybir.dt.float32
f32r = mybir.dt.float32r
fp16 = mybir.dt.float16
AF = mybir.ActivationFunctionType

_CACHE = {}


def _build():
    if "nc" in _CACHE:
        return _CACHE["nc"]
    nc = bacc.Bacc("TRN2", target_bir_lowering=False, debug=False, num_devices=8)
    qt = nc.dram_tensor("qt", [D, S], f32r, kind="ExternalInput").ap()
    kt = nc.dram_tensor("kt", [D, S], f32r, kind="ExternalInput").ap()
    vt = nc.dram_tensor("vt", [D, S], f32r, kind="ExternalInput").ap()
    wqt = nc.dram_tensor("wqt", [D, HD], f32r, kind="ExternalInput").ap()
    wkt = nc.dram_tensor("wkt", [D, HD], f32r, kind="ExternalInput").ap()
    wvt = nc.dram_tensor("wvt", [D, HD], f32r, kind="ExternalInput").ap()
    wot = nc.dram_tensor("wot", [HD, D], fp16, kind="ExternalInput").ap()
    y = nc.dram_tensor("y", [S, D], f32, kind="ExternalOutput").ap()

    with tile.TileContext(nc) as tc:
        with (
            tc.tile_pool(name="wp", bufs=3) as wp,
            tc.tile_pool(name="wop", bufs=1) as wop,
            tc.tile_pool(name="xp", bufs=8) as xp,
            tc.tile_pool(name="qk", bufs=2 * MT) as qkp,
            tc.tile_pool(name="vg", bufs=ST) as vgp,
            tc.tile_pool(name="ot", bufs=MT) as otp,
            tc.tile_pool(name="ep", bufs=8) as epp,
            tc.tile_pool(name="sm", bufs=2) as smp,
            tc.tile_pool(name="yp", bufs=2) as ypp,
        ):
            # ---- persistent SBUF tensors ----
            qht = [qkp.tile([128, S], fp16, tag="qk", name=f"qht{i}") for i in range(MT)]
            kht = [qkp.tile([128, S], fp16, tag="qk", name=f"kht{i}") for i in range(MT)]
            vaug = [vgp.tile([128, HPC, DEPTH + 1], fp16, tag="vg", name=f"vaug{i}") for i in range(ST)]
            outt = [otp.tile([128, S], fp16, tag="ot", name=f"outt{i}") for i in range(MT)]

            wot_sb = wop.tile([128, MT, D], fp16, tag="wot")
            nc.sync.dma_start(
                out=wot_sb[:], in_=wot.rearrange("(m p) o -> p m o", p=128)
            )

            def load_w(wdram, nm):
                w_sb = wp.tile([128, EC, HD], f32r, tag="w", name=f"w_{nm}")
                for ci in range(EC):
                    nc.sync.dma_start(
                        out=w_sb[:, ci, :],
                        in_=wdram[ci * 128 : (ci + 1) * 128, :],
                    )
                return w_sb

            def load_x(xdram, sh, nm):
                xc = [
                    xp.tile([128, S // SH], f32r, tag="x", name=f"x{nm}{sh}_{i}")
                    for i in range(EC)
                ]
                for ci in range(EC):
                    nc.sync.dma_start(
                        out=xc[ci][:],
                        in_=xdram[
                            ci * 128 : (ci + 1) * 128,
                            sh * (S // SH) : (sh + 1) * (S // SH),
                        ],
                    )
                return xc

            with (
                tc.tile_pool(name="plog", bufs=2, space="PSUM") as plog,
                tc.tile_pool(name="pacc", bufs=2, space="PSUM") as pacc,
            ):

                def proj_qk_full(name, xdram, w_sb, dst):
                    # sh-outer, m-inner: x halves loaded once, all m passes
                    for sh in range(SH):
                        xc = load_x(xdram, sh, name)
                        for m in range(MT):
                            pt = plog.tile(
                                [128, S // SH], f32, tag="plog", name=f"p{name}{sh}_{m}"
                            )
                            for ci in range(EC):
                                for n in range(S // SH // 512):
                                    nc.tensor.matmul(
                                        pt[:, n * 512 : (n + 1) * 512],
                                        w_sb[:, ci, m * 128 : (m + 1) * 128],
                                        xc[ci][:, n * 512 : (n + 1) * 512],
                                        start=(ci == 0),
                                        stop=(ci == EC - 1),
                                    )
                            with nc.allow_low_precision(reason="fp16 attention"):
                                nc.vector.tensor_copy(
                                    dst[m][:, sh * (S // SH) : (sh + 1) * (S // SH)],
                                    pt[:],
                                )

                def proj_v():
                    wv_sb = load_w(wvt, "v")
                    for sh in range(SH):
                        xc = load_x(vt, sh, "v")
                        for st in range(ST // SH):
                            s = sh * (ST // SH) + st
                            pv = pacc.tile([128, HD], f32, tag="pacc", name=f"pv{s}")
                            for ci in range(EC):
                                nc.tensor.matmul(
                                    pv[:],
                                    xc[ci][:, st * 128 : (st + 1) * 128],
                                    wv_sb[:, ci, :],
                                    start=(ci == 0),
                                    stop=(ci == EC - 1),
                                )
                            with nc.allow_low_precision(reason="fp16 attention"):
                                nc.vector.tensor_copy(
                                    vaug[s][:, :, 0:DEPTH],
                                    pv[:].rearrange("p (h d) -> p h d", d=DEPTH),
                                )
                            nc.vector.memset(vaug[s][:, :, DEPTH : DEPTH + 1], 1.0)

                def attn(h, qh):
                    m = h // 2
                    base = (h % 2) * 64
                    q0 = qh * QHS
                    acc = pacc.tile(
                        [DEPTH + 1, QHS], f32, tag="pacc", name=f"acc{h}_{qh}"
                    )
                    for kt_i in range(ST):
                        lp = plog.tile(
                            [128, QHS], f32, tag="plog", name=f"lp{h}_{qh}_{kt_i}"
                        )
                        for n in range(QHS // 512):
                            nc.tensor.matmul(
                                lp[:, n * 512 : (n + 1) * 512],
                                kht[m][
                                    base : base + 64,
                                    kt_i * 128 : (kt_i + 1) * 128,
                                ],
                                qht[m][
                                    base : base + 64,
                                    q0 + n * 512 : q0 + (n + 1) * 512,
                                ],
                                start=True,
                                stop=True,
                            )
                        et = epp.tile(
                            [128, QHS], fp16, tag="ep", name=f"et{h}_{qh}_{kt_i}"
                        )
                        with nc.allow_low_precision(reason="fp16 attention"):
                            nc.scalar.activation(
                                et[:], lp[:], AF.Exp, scale=1.0 / np.sqrt(DEPTH)
                            )
                        for n in range(QHS // 512):
                            nc.tensor.matmul(
                                acc[:, n * 512 : (n + 1) * 512],
                                vaug[kt_i][:, h, :],
                                et[:, n * 512 : (n + 1) * 512],
                                start=(kt_i == 0),
                                stop=(kt_i == ST - 1),
                            )
                    r = smp.tile([1, QHS], f32, tag="r", name=f"r{h}_{qh}")
                    nc.vector.reciprocal(r[:], acc[DEPTH : DEPTH + 1, :])
                    rb = smp.tile([64, QHS], f32, tag="rb", name=f"rb{h}_{qh}")
                    nc.gpsimd.partition_broadcast(rb[:], r[:])
                    with nc.allow_low_precision(reason="fp16 attention"):
                        nc.vector.tensor_mul(
                            outt[m][base : base + 64, q0 : q0 + QHS],
                            acc[0:DEPTH, :],
                            rb[:],
                        )

                def outproj(s):
                    py = pacc.tile([128, D], f32, tag="pacc", name=f"py{s}")
                    for m in range(MT):
                        for n0, n1 in ((0, 512), (512, 768)):
                            nc.tensor.matmul(
                                py[:, n0:n1],
                                outt[m][:, s * 128 : (s + 1) * 128],
                                wot_sb[:, m, n0:n1],
                                start=(m == 0),
                                stop=(m == MT - 1),
                            )
                    ty = ypp.tile([128, D], f32, tag="y", name=f"ty{s}")
                    nc.vector.tensor_copy(ty[:], py[:])
                    nc.sync.dma_start(out=y[s * 128 : (s + 1) * 128, :], in_=ty[:])

                # ---- v2-style phase emission ----
                proj_v()
                wq_sb = load_w(wqt, "q")
                proj_qk_full("q", qt, wq_sb, qht)
                wk_sb = load_w(wkt, "k")
                proj_qk_full("k", kt, wk_sb, kht)
                for h in range(HPC):
                    attn(h, 0)
                    attn(h, 1)
                for s in range(ST):
                    outproj(s)

    nc.compile()
    _CACHE["nc"] = nc
    return nc


def _round_f32r(x: np.ndarray) -> np.ndarray:
    b = np.ascontiguousarray(x, dtype=np.float32).view(np.uint32).astype(np.int64)
    r = (b + 0x800 + ((b >> 12) & 1)) & ~0xFFF
    return r.astype(np.uint32).view(np.float32)


def make_in_maps(v, k, q, wq, wk, wv, wo):
    in_maps = []
    for c in range(8):
        b = c // 2
        hs = (c % 2) * HD
        in_maps.append(
            {
                "qt": _round_f32r(q[b].T),
                "kt": _round_f32r(k[b].T),
                "vt": _round_f32r(v[b].T),
                "wqt": _round_f32r(wq[hs : hs + HD, :].T),
                "wkt": _round_f32r(wk[hs : hs + HD, :].T),
                "wvt": _round_f32r(wv[hs : hs + HD, :].T),
                "wot": np.ascontiguousarray(
                    wo[:, hs : hs + HD].T, dtype=np.float32
                ).astype(np.float16),
            }
        )
    return in_maps


def assemble(results, bo):
    y = np.empty((B, S, D), dtype=np.float32)
    for b in range(B):
        y[b] = results[2 * b]["y"] + results[2 * b + 1]["y"] + bo[None, :]
    return y


def kernel(v, k, q, wq, wk, wv, wo, bo):
    nc = _build()
    in_maps = make_in_maps(v, k, q, wq, wk, wv, wo)
    res = run_bass_kernel_spmd(nc, in_maps, list(range(8)))
    return assemble(res.results, np.asarray(bo, dtype=np.float32))
